# revision 52
# baseline (speedup 1.0000x reference)
"""HEPOS BART cross-attention Trainium2 kernel (bf16, PE-dense rewrite).

Shapes (hardcoded): B=2, Tq=1024, Tk=8192, E=1024, H=16, D=64, stride=16,
m = Tk//stride = 512 keys per head.

Sharding: 8 cores = 2 batches x 4 head-groups (4 heads each).
All DMA'd tensors are bf16 (halves HBM traffic vs fp32; PE rate is the same
1 cycle/row as fp32r). PSUM accumulation is fp32 throughout.

Per core:
  phase 1: QT = (Wq_hg @ hs_b^T)*scale + bq  -> qt[h] [64, 1024] bf16
           (4 sequential psum groups of 8 matmuls; bias added on DVE)
  phase 2a per head: fused KV proj: stationary [Wk_h | Wv_h] chunk [128,128]
           -> psum [128, 512]: rows 0-63 KgT, 64-127 VgT. One DVE copy to
           SBUF; 4 PE transposes of the V half -> vgp [128, 4, 65]
           (col 64 = ones -> softmax denominator comes free in attn@V).
  phase 2b (tqt outer, h inner, scores pipelined one head ahead of attn@V):
           scores: 4 matmuls into 2x [128, 2, 512] psum tiles
           exp: 2 scalar activations [128, 1024] -> expT bf16
           attn@V: 4 matmuls accum -> ps_ov [65, 512] (row 64 = Z)
           rinv = 1/Z: native DVE copy of the Z row to SBUF partition 0
           (the custom-DVE recip misreads partition-shifted PSUM on HW),
           then DVE reciprocal_approx_fast; broadcast across partitions on
           GpSimd; normalize-mult on DVE, deferred one stage so the DVE
           queue never stalls on the GpSimd broadcast.
  phase 3 (per tqt half, interleaved with 2b): partial = outT^T @ WoT
           -> psum -> SBUF (copy split scalar/DVE) -> DMA.
Scalar runs ONLY {Exp, Copy, Identity} so a single ACT table load suffices
(Exp and Reciprocal tables cannot coexist; switching costs 1.3us each).
bk is dropped (constant key shift cancels in softmax); bv folded into the
host-side final bias (bv @ Wo.T + bo).
"""

import numpy as np
import ml_dtypes

import concourse.bass as bass
import concourse.bacc as bacc
import concourse.tile as tile
from concourse import library_config, mybir
from concourse.masks import make_identity

B, Tq, Tk, E, H, D = 2, 1024, 8192, 1024, 16, 64
STRIDE = 16
M = Tk // STRIDE          # 512 keys per head
HPC = 4                   # heads per core
NCORES = 8
F32 = mybir.dt.float32
BF16 = mybir.dt.bfloat16
NPBF16 = ml_dtypes.bfloat16


def build_program():
    nc = bacc.Bacc("TRN2", target_bir_lowering=False)

    # dram tensors already in SBUF layout
    # hsT col = tqt*4096 + e*512 + t ; wq col = pair*1024 + e*128 + j
    hsT = nc.dram_tensor("hsT", [128, 8 * Tq], BF16, kind="ExternalInput")
    wq = nc.dram_tensor("wq", [128, 2048], BF16, kind="ExternalInput")
    wkv = nc.dram_tensor("wkv", [128, 4096], BF16, kind="ExternalInput")
    kvg = nc.dram_tensor("kvg", [HPC, 128, 8 * M], BF16, kind="ExternalInput")
    wo = nc.dram_tensor("wo", [128, 2048], BF16, kind="ExternalInput")
    bqp = nc.dram_tensor("bqp", [2, 128, 1], F32, kind="ExternalInput")
    out = nc.dram_tensor("out", [16, 128, 512], BF16, kind="ExternalOutput")

    _dma_engs = [None, None]
    _dma_i = [0]

    def dma(out_ap, in_ap):
        eng = _dma_engs[_dma_i[0] % 2]
        _dma_i[0] += 1
        eng.dma_start(out=out_ap, in_=in_ap)

    with tile.TileContext(nc) as tc:
        _dma_engs[0] = nc.sync
        _dma_engs[1] = nc.gpsimd
        with (
            tc.tile_pool(name="consts", bufs=1) as consts,
            tc.tile_pool(name="expp", bufs=3) as expp,
            tc.tile_pool(name="rowp", bufs=3) as rowp,
            tc.tile_pool(name="pop", bufs=4) as pop,
            tc.tile_pool(name="psA", bufs=3, space="PSUM") as psA,
            tc.tile_pool(name="psB", bufs=2, space="PSUM") as psB,
        ):
            # ---- persistent SBUF tiles -------------------------------------
            hsT_sb = consts.tile([128, 8 * Tq], BF16)
            wq_sb = consts.tile([128, 2048], BF16)
            wkv_sb = consts.tile([128, 4096], BF16)
            wo_sb = consts.tile([128, 2048], BF16)
            kvg_sb = [consts.tile([128, 8 * M], BF16, name=f"kvg{h}")
                      for h in range(HPC)]
            kv_sb = [consts.tile([128, M], BF16, name=f"kv{h}")
                     for h in range(HPC)]
            vgp_sb = [consts.tile([128, 4, D + 1], BF16, name=f"vgp{h}")
                      for h in range(HPC)]
            qt_sb = [consts.tile([D, Tq], BF16, name=f"qt{h}")
                     for h in range(HPC)]
            outT_sb = [consts.tile([128, Tq], BF16, name=f"outT{dd}")
                       for dd in range(2)]
            bq_sb = [consts.tile([128, 1], F32, name=f"bq{p}") for p in range(2)]

            identf = consts.tile([128, 128], F32)
            make_identity(nc, identf)
            ident = consts.tile([128, 128], BF16)
            nc.vector.tensor_copy(ident[:], identf[:])
            for h in range(HPC):
                nc.vector.memset(vgp_sb[h][:, :, D:D + 1], 1.0)

            # ---- input DMAs, two waves ------------------------------------
            # Wave 1 (phase1 + head0) gets the queues to itself so its bytes
            # land early; later waves are gated behind it via tiny gpsimd
            # reads (DMA queues round-robin between all enqueued transfers,
            # so an ungated bulk enqueue starves the critical path).
            gate_sb = consts.tile([1, 8], BF16)
            _gate_i = [0]

            def gate(sl):
                # 4-byte SBUF->SBUF DMA on sync whose read-dep stalls sync
                # (and thus all later sync dispatches) until `sl`'s writer
                # DMA has landed.
                i = _gate_i[0]
                _gate_i[0] += 1
                nc.sync.dma_start(out=gate_sb[:, i:i + 1], in_=sl)

            # wave 1 (ungated): tqt0 phase-1 data + head-0 K/V
            dma(wq_sb[:, 0:1024], wq[:, 0:1024])                   # sync
            dma(hsT_sb[:, 0:2048], hsT[:, 0:2048])                 # gpsimd
            dma(hsT_sb[:, 2048:4096], hsT[:, 2048:4096])           # sync
            dma(wq_sb[:, 1024:2048], wq[:, 1024:2048])             # gpsimd
            dma(kvg_sb[0][:, 0:2048], kvg[0][:, 0:2048])           # sync
            dma(kvg_sb[0][:, 2048:4096], kvg[0][:, 2048:4096])     # gpsimd
            dma(wkv_sb[:, 0:1024], wkv[:, 0:1024])                 # sync
            dma(bq_sb[0][:], bqp[0])
            dma(bq_sb[1][:], bqp[1])
            # waves 2..5 gated so each head's bytes land in need order;
            # the tqt1 hsT halves ride along in waves 2/3
            gate(kvg_sb[0][127:128, 4095:4096])
            gate(hsT_sb[127:128, 4095:4096])
            nc.sync.dma_start(out=kvg_sb[1][:, 0:2048],
                              in_=kvg[1][:, 0:2048])
            nc.sync.dma_start(out=kvg_sb[1][:, 2048:4096],
                              in_=kvg[1][:, 2048:4096])
            nc.sync.dma_start(out=wkv_sb[:, 1024:2048],
                              in_=wkv[:, 1024:2048])
            nc.sync.dma_start(out=hsT_sb[:, 4096:6144],
                              in_=hsT[:, 4096:6144])
            gate(kvg_sb[1][127:128, 4095:4096])
            nc.sync.dma_start(out=kvg_sb[2][:, 0:2048],
                              in_=kvg[2][:, 0:2048])
            nc.sync.dma_start(out=kvg_sb[2][:, 2048:4096],
                              in_=kvg[2][:, 2048:4096])
            nc.sync.dma_start(out=wkv_sb[:, 2048:3072],
                              in_=wkv[:, 2048:3072])
            nc.sync.dma_start(out=hsT_sb[:, 6144:8192],
                              in_=hsT[:, 6144:8192])
            gate(kvg_sb[2][127:128, 4095:4096])
            nc.sync.dma_start(out=kvg_sb[3][:, 0:2048],
                              in_=kvg[3][:, 0:2048])
            nc.sync.dma_start(out=kvg_sb[3][:, 2048:4096],
                              in_=kvg[3][:, 2048:4096])
            nc.sync.dma_start(out=wkv_sb[:, 3072:4096],
                              in_=wkv[:, 3072:4096])
            gate(kvg_sb[3][127:128, 4095:4096])
            nc.sync.dma_start(out=wo_sb[:, 0:1024], in_=wo[:, 0:1024])
            nc.sync.dma_start(out=wo_sb[:, 1024:2048],
                              in_=wo[:, 1024:2048])

            # ---- stage closures (issued in an interleaved order so the
            # in-order PE queue never head-of-line blocks on a late DMA) ----
            def ph1(pair, tqt):
                ps_qt = psA.tile([128, 2, 512], F32, tag="A", name="ps_qt")
                for e in range(8):
                    nc.tensor.matmul(
                        ps_qt[:, 0, :],
                        wq_sb[:, pair * 1024 + e * 128:
                              pair * 1024 + (e + 1) * 128],
                        hsT_sb[:, tqt * 4096 + e * 512:
                               tqt * 4096 + (e + 1) * 512],
                        start=(e == 0), stop=(e == 7))
                for sub in range(2):
                    h = 2 * pair + sub
                    nc.vector.tensor_scalar_add(
                        qt_sb[h][:, tqt * 512: tqt * 512 + 512],
                        ps_qt[sub * 64:(sub + 1) * 64, 0, :],
                        bq_sb[pair][sub * 64:(sub + 1) * 64, 0:1])

            def p2a(h):
                ps_kv = psA.tile([128, 2, 512], F32, tag="A", name="ps_kv")
                for e in range(8):
                    nc.tensor.matmul(
                        ps_kv[:, 0, :],
                        wkv_sb[:, (h * 8 + e) * 128:(h * 8 + e + 1) * 128],
                        kvg_sb[h][:, e * M:(e + 1) * M],
                        start=(e == 0), stop=(e == 7))
                nc.scalar.copy(kv_sb[h][:], ps_kv[:, 0, :])
                ps_vt = psB.tile([128, 4, D], BF16, tag="B", name="ps_vt")
                for mc in range(4):
                    nc.tensor.transpose(
                        ps_vt[:, mc, :],
                        kv_sb[h][64:128, mc * 128:(mc + 1) * 128],
                        ident[64:128, 64:128])
                nc.vector.tensor_copy(vgp_sb[h][:, :, 0:D], ps_vt[:])

            pending = []

            def flush():
                while pending:
                    pending.pop(0)()

            def sc_part(tqt, h):
                expT = expp.tile([128, 4, 512], BF16, tag="expT", name="expT")
                for mcp in range(2):
                    ps_sc = psA.tile([128, 2, 512], F32, tag="A", name="ps_sc")
                    for sub in range(2):
                        mc = 2 * mcp + sub
                        nc.tensor.matmul(
                            ps_sc[:, sub, :],
                            kv_sb[h][0:D, mc * 128:(mc + 1) * 128],
                            qt_sb[h][:, tqt * 512: tqt * 512 + 512],
                            start=True, stop=True)
                    nc.scalar.activation(
                        expT[:, 2 * mcp:2 * mcp + 2, :],
                        ps_sc[:],
                        mybir.ActivationFunctionType.Exp)
                return expT

            def av_part(tqt, h, expT):
                if pending:
                    pending.pop(0)()
                ps_ov = psB.tile([128, 512], F32, tag="B", name="ps_ov")
                for mc in range(4):
                    nc.tensor.matmul(
                        ps_ov[0:D + 1, :],
                        vgp_sb[h][:, mc, :],
                        expT[:, mc, :],
                        start=(mc == 0), stop=(mc == 3))
                # rinv = 1/Z on DVE (fast approx), broadcast on GpSimd.
                # Z must reach SBUF partition 0 via a native op first: the
                # custom-DVE recip misreads partition-shifted PSUM on HW.
                zrow = rowp.tile([1, 512], F32, tag="zrow", name="zrow")
                nc.vector.tensor_copy(zrow[:], ps_ov[D:D + 1, :])
                rinv = rowp.tile([1, 512], F32, tag="rinv", name="rinv")
                nc.vector.reciprocal_approx_fast(rinv[:], zrow[:])
                rinv_b = rowp.tile([D, 512], F32, tag="rinv_b", name="rinv_b")
                nc.gpsimd.partition_broadcast(rinv_b[:], rinv[:])

                # defer the normalize-mult by one stage so the vector queue
                # doesn't stall waiting on the gpsimd broadcast
                def _mult(tqt=tqt, h=h, ps_ov=ps_ov, rinv_b=rinv_b):
                    nc.vector.tensor_tensor(
                        outT_sb[h // 2][(h % 2) * D:(h % 2 + 1) * D,
                                        tqt * 512: tqt * 512 + 512],
                        ps_ov[0:D, :],
                        rinv_b[:],
                        op=mybir.AluOpType.mult)
                pending.append(_mult)

            def po(t8):
                flush()
                ps_po = psA.tile([128, 2, 512], F32, tag="A", name="ps_po")
                for eot in range(2):
                    for dd in range(2):
                        nc.tensor.matmul(
                            ps_po[:, eot, :],
                            outT_sb[dd][:, t8 * 128:(t8 + 1) * 128],
                            wo_sb[:, dd * E + eot * 512:
                                  dd * E + eot * 512 + 512],
                            start=(dd == 0), stop=(dd == 1))
                po_sb = pop.tile([128, 2, 512], BF16, tag="po", name="po_sb")
                nc.scalar.copy(po_sb[:, 0, :], ps_po[:, 0, :])
                nc.vector.tensor_copy(po_sb[:, 1, :], ps_po[:, 1, :])
                nc.sync.dma_start(out=out[t8 * 2], in_=po_sb[:, 0, :])
                nc.sync.dma_start(out=out[t8 * 2 + 1], in_=po_sb[:, 1, :])

            ph1(0, 0)
            ph1(1, 0)
            p2a(0)
            e00 = sc_part(0, 0)
            av_part(0, 0, e00)
            p2a(1)
            e01 = sc_part(0, 1)
            av_part(0, 1, e01)
            p2a(2)
            e02 = sc_part(0, 2)
            av_part(0, 2, e02)
            p2a(3)
            e03 = sc_part(0, 3)
            av_part(0, 3, e03)
            ph1(0, 1)
            ph1(1, 1)
            e10 = sc_part(1, 0)
            av_part(1, 0, e10)
            po(0)
            po(1)
            e11 = sc_part(1, 1)
            av_part(1, 1, e11)
            e12 = sc_part(1, 2)
            av_part(1, 2, e12)
            e13 = sc_part(1, 3)
            av_part(1, 3, e13)
            po(2)
            po(3)
            for t8 in range(4, 8):
                po(t8)

    nc.compile()
    return nc


_NC = None


def _get_nc():
    global _NC
    if _NC is None:
        _NC = build_program()
    return _NC


def shard_inputs(hidden_states, key_value_states, Wq, bq, Wk, bk, Wv, bv, Wo,
                 bo, stride):
    stride = int(stride)
    assert stride == STRIDE
    scale = float(D) ** -0.5
    bf = lambda a: np.ascontiguousarray(a).astype(NPBF16)
    in_maps = []
    for c in range(NCORES):
        b, g = divmod(c, 4)
        h0 = g * HPC
        r0, r1 = h0 * D, (h0 + HPC) * D
        # hsT [128, 8192]: col tqt*4096+e*512+t = hs[b][tqt*512+t, e*128+p]
        hsT_c = hidden_states[b].T.reshape(8, 128, 2, 512) \
            .transpose(1, 2, 0, 3).reshape(128, 8 * Tq)
        # wq [128, 2048]: col pair*1024+e*128+j = Wq_s[pair*128+j, e*128+p]
        Wqs = (Wq[r0:r1, :] * scale)
        wq_c = Wqs.T.reshape(8, 128, 2, 128).transpose(1, 2, 0, 3) \
            .reshape(128, 2048)
        # wkv [128, 4096]: col (h*8+e)*128+j: j<64 Wk, j>=64 Wv
        K3 = Wk[r0:r1, :].reshape(HPC, D, E)
        V3 = Wv[r0:r1, :].reshape(HPC, D, E)
        C = np.concatenate([K3, V3], axis=1)          # [4, 128, 1024]
        wkv_c = C.reshape(4, 128, 8, 128).transpose(3, 0, 2, 1) \
            .reshape(128, 4096)
        # kvg [4, 128, 4096]: col e*512+m = kv[b, h0+h+16m, e*128+p]
        kvg_c = np.empty((HPC, 128, 8 * M), NPBF16)
        for hl in range(HPC):
            R = key_value_states[b, (h0 + hl)::STRIDE, :]     # [512, 1024]
            kvg_c[hl] = bf(R.T.reshape(8, 128, M).transpose(1, 0, 2)
                           .reshape(128, 8 * M))
        # wo [128, 2048]: col dd*1024+n = Wo[n, r0+dd*128+p]
        wo_c = Wo[:, r0:r1].T.reshape(2, 128, E).transpose(1, 0, 2) \
            .reshape(128, 2048)
        bqp_c = (bq[r0:r1] * scale).astype(np.float32).reshape(2, 128, 1)
        in_maps.append({
            "hsT": bf(hsT_c),
            "wq": bf(wq_c),
            "wkv": bf(wkv_c),
            "kvg": np.ascontiguousarray(kvg_c),
            "wo": bf(wo_c),
            "bqp": bqp_c,
        })
    return in_maps


def combine_outputs(results, Wv, bv, Wo, bo):
    final_bias = (bv @ Wo.T + bo).astype(np.float32)  # [E]
    out = np.zeros((B, Tq, E), np.float32)
    for c in range(NCORES):
        b = c // 4
        # out dram [16, 128, 512]: chunk t8*2+eot
        o = results[c]["out"].astype(np.float32) \
            .reshape(8, 2, 128, 512).transpose(0, 2, 1, 3).reshape(Tq, E)
        out[b] += o
    out += final_bias[None, None, :]
    return out


def kernel(hidden_states, key_value_states, Wq, bq, Wk, bk, Wv, bv, Wo, bo,
           stride, _trace=False, _trace_kwargs=None):
    from concourse.bass_utils import run_bass_kernel_spmd

    args = [np.asarray(x, np.float32) for x in
            (hidden_states, key_value_states, Wq, bq, Wk, bk, Wv, bv, Wo, bo)]
    (hidden_states, key_value_states, Wq, bq, Wk, bk, Wv, bv, Wo, bo) = args
    in_maps = shard_inputs(hidden_states, key_value_states, Wq, bq, Wk, bk,
                           Wv, bv, Wo, bo, stride)
    nc = _get_nc()
    res = run_bass_kernel_spmd(
        nc, in_maps, list(range(NCORES)),
        trace=_trace, **(_trace_kwargs or {}))
    out = combine_outputs(res.results, Wv, bv, Wo, bo)
    kernel.last_run = res
    return out
